# revision 19
# baseline (speedup 1.0000x reference)
"""BERT self-attention (B=4, S=1024, H=1024, 16 heads, d=64) on 8 TRN2 cores.

Sharding: core c = b*2 + g handles batch b and head-group g (8 heads, 512
output columns).  No cross-core communication: each core gets its batch's
hidden_states plus the column slice of Wq/Wk/Wv for its head group, and
produces out[b, :, g*512:(g+1)*512].

Per-core dataflow (matmul inputs fp16, accumulation fp32 PSUM):
  1. X^T comes pre-transposed from the host; ct=0 W slices load first so
     the Q/K projections start as X chunks stream in.
  2. Head-PAIR loop: the two heads of column-block ct live on partition
     halves 0-63 / 64-127 of qt/kt, so their scores matmuls (K=64) carry
     tile_position (0,0) / (64,0) and run CONCURRENTLY on the PE array
     when issued back-to-back -> scores effectively 2x.
  3. One [128,2048] exp per kt chunk covers both heads and both q halves
     (fewer ACT startups).  Vaug carries a ones column so ctx~^T =
     Vaug^T P^T also yields softmax denominators in partition 64.
  4. ctx^T [65,512] is cast to fp16 (PSUM evacuation), PE-transposed back
     to [q, d+1]; recip of the denominator column is [128,4] (cheap), and
     one stride-0-broadcast tensor_tensor multiply normalizes.  PE never
     waits on more than the single evacuation cast.
"""

import numpy as np

B, S, H = 4, 1024, 1024
NH, D = 16, 64
NCORES = 8
HG = NH // 2        # heads per core
CW = HG * D         # output columns per core (512)
P = 128             # partitions

_CACHE = {}


def _split_excess_waits(nc, mybir):
    """Walrus codegen allows 1 sync-wait per instruction (2 for
    EventSemaphore); Tile's tail drain (and some matmuls) carry more.
    Move the excess onto NoOp carriers inserted just before, same engine."""
    for f in nc.m.functions:
        for bb in f.blocks:
            new_insts, changed = [], False
            for inst in bb.instructions:
                si = inst.sync_info
                cap = 2 if inst.opcode == "EventSemaphore" else 1
                if si is not None and si.on_wait and len(si.on_wait) > cap:
                    waits = list(si.on_wait)
                    for i, w in enumerate(waits[:-cap]):
                        nop = mybir.InstNoOp(
                            name=f"{inst.name}-wsplit{i}",
                            engine=inst.engine,
                            sync_info=mybir.SyncInfo(on_wait=[w], on_update=[]),
                            bass_nofuse=True,
                        )
                        nc.register_instruction(nop, overwrite=True)
                        new_insts.append(nop)
                    inst.sync_info = mybir.SyncInfo(
                        on_wait=waits[-cap:],
                        on_update=list(si.on_update or []))
                    changed = True
                new_insts.append(inst)
            if changed:
                bb.instructions = new_insts


def _build():
    import concourse.bass as bass
    import concourse.mybir as mybir
    import concourse.tile as tile
    from contextlib import ExitStack

    f32 = mybir.dt.float32
    f16 = mybir.dt.float16
    EXP = mybir.ActivationFunctionType.Exp

    nc = bass.Bass()
    x_d = nc.dram_tensor("x", [H, S], f16, kind="ExternalInput")  # X^T
    wqk_d = nc.dram_tensor("wqk", [4, P, 2, 8, P], f16,
                           kind="ExternalInput")  # [ct, p, {q,k}, hcc, col]
    wv_d = nc.dram_tensor("wv", [H, CW], f16, kind="ExternalInput")
    bq_d = nc.dram_tensor("bq", [P, 4], f32, kind="ExternalInput")
    bk_d = nc.dram_tensor("bk", [P, 4], f32, kind="ExternalInput")
    bvb_d = nc.dram_tensor("bvb", [P, CW], f32, kind="ExternalInput")
    id16_d = nc.dram_tensor("id16", [P, P], f16, kind="ExternalInput")
    out_d = nc.dram_tensor("out", [NH // 2, P, 2, 4, D], f16,
                           kind="ExternalOutput")  # [h,p,qb,j,d]

    with tile.TileContext(nc) as tc, ExitStack() as ctx:
        persist = ctx.enter_context(tc.tile_pool(name="persist", bufs=1))
        ptpool = ctx.enter_context(tc.tile_pool(name="ptpool", bufs=3))
        rpool = ctx.enter_context(tc.tile_pool(name="rpool", bufs=4))
        opool = ctx.enter_context(tc.tile_pool(name="opool", bufs=4))
        pss = ctx.enter_context(tc.tile_pool(name="pss", bufs=1, space="PSUM"))
        psc = ctx.enter_context(tc.tile_pool(name="psc", bufs=3, space="PSUM"))
        pst = ctx.enter_context(tc.tile_pool(name="pst", bufs=1, space="PSUM"))

        wqk_s = persist.tile([P, 4, 2, 8, P], f16, tag="wqk")
        wv_s = persist.tile([P, 8, CW], f16, tag="wv")

        xt = persist.tile([P, 8, S], f16, tag="xt")          # X^T [h, hc, s]
        qt = persist.tile([P, 4, S], f16, tag="qt")          # Q^T [col, ct, s]
        kt = persist.tile([P, 4, S], f16, tag="kt")          # K^T
        vaug = persist.tile([P, 8, HG, D + 1], f16, tag="vaug")  # V + ones col
        bqs = persist.tile([P, 4], f32, tag="bqs")
        bks = persist.tile([P, 4], f32, tag="bks")
        bvb = persist.tile([P, CW], f32, tag="bvb")          # bv broadcast
        ident = persist.tile([P, P], f16, tag="ident")
        actw = persist.tile([P, 8], f32, tag="actw")         # exp table warm
        onesf = persist.tile([P, 8, HG], f16, tag="onesf")

        # ---- input DMAs: ct0 W slices first (small), X split across both
        # HWDGE queues, then wv, then the remaining W slices ----
        # X split across BOTH HWDGE rings; only 4 trigger instructions sit
        # on the scalar engine so the ACT NX reaches the exp stream quickly.
        # W comes as host-prepped contiguous per-ct blocks (2KB/partition
        # runs) -- strided 256B-descriptor loads in front of the ring would
        # delay every X chunk behind them.
        x_r = x_d.rearrange("(c p) s -> p c s", p=P)
        nc.sync.dma_start(out=wqk_s[:, 0], in_=wqk_d[0])
        nc.sync.dma_start(out=bqs, in_=bq_d[:, :])
        nc.sync.dma_start(out=bks, in_=bk_d[:, :])
        for hc in (0, 1):
            nc.sync.dma_start(out=xt[:, hc, :], in_=x_r[:, hc, :])
        for hc in (2, 3, 4, 5, 6, 7):
            nc.scalar.dma_start(out=xt[:, hc, :], in_=x_r[:, hc, :])
        nc.sync.dma_start(out=wqk_s[:, 1], in_=wqk_d[1])
        nc.sync.dma_start(out=wv_s, in_=wv_d.rearrange("(c p) n -> p c n", p=P))
        nc.sync.dma_start(out=ident, in_=id16_d[:, :])
        nc.sync.dma_start(out=bvb, in_=bvb_d[:, :])
        nc.sync.dma_start(out=wqk_s[:, 2], in_=wqk_d[2])
        nc.sync.dma_start(out=wqk_s[:, 3], in_=wqk_d[3])
        nc.vector.memset(onesf, 1.0)
        nc.vector.tensor_copy(vaug[:, :, :, D], onesf)
        warm = persist.tile([P, 512], f16, tag="warm")
        nc.vector.memset(warm, 0.0)
        ps_w = pss.tile([P, 512], f32, tag="pss", name="ps_w")
        for i in range(26):
            nc.tensor.matmul(ps_w[0:HG, :], lhsT=onesf[:, 0, :], rhs=warm,
                             start=True, stop=True)
        # touch Exp on ACT immediately so the ~2.7us table load hides
        # inside the DMA ramp instead of delaying the first real exp
        nc.scalar.activation(actw[0:1, :], onesf[0:1, 0, 0:8], EXP)

        def emit_qtkt_chunk(ct, wi, sb):
            """One (weight, s-half) chunk of the Q/K projection: 8 accum
            matmuls [128,512] + bias-add to qt/kt fp16."""
            b_s, dst = ((bqs, qt), (bks, kt))[wi]
            ps = psc.tile([P, 512], f32, tag="psc")
            for hcc in range(8):
                nc.tensor.matmul(
                    ps,
                    lhsT=wqk_s[:, ct, wi, hcc, :],
                    rhs=xt[:, hcc, sb * 512:(sb + 1) * 512],
                    start=(hcc == 0), stop=(hcc == 7))
            nc.vector.tensor_scalar_add(
                dst[:, ct, sb * 512:(sb + 1) * 512], ps, b_s[:, ct:ct + 1])

        def emit_v_chunk(st):
            ps = psc.tile([P, 512], f32, tag="psc")
            for hcc in range(8):
                nc.tensor.matmul(
                    ps,
                    lhsT=xt[:, hcc, st * P:(st + 1) * P],
                    rhs=wv_s[:, hcc, :],
                    start=(hcc == 0), stop=(hcc == 7))
            nc.vector.tensor_add(
                vaug[:, st, :, 0:D],
                ps.rearrange("p (h d) -> p h d", h=HG),
                bvb.rearrange("p (h d) -> p h d", h=HG))

        pt_of = {}

        def emit_scores4(ct, kt_i):
            """Both heads of ct, both q halves, one kt chunk: 4 matmuls.
            Head-even uses partitions 0:64 (tile_position (0,0)), head-odd
            64:128 ((64,0)) -> adjacent pairs run concurrently on the PE.
            One [128,2048] exp covers all four results."""
            if kt_i == 0:
                pt_of[ct] = ptpool.tile([P, 2, 8, S], f16, tag="ptp",
                                        name=f"ptp{ct}")
            ptp = pt_of[ct]
            ps = pss.tile([P, 4, 512], f32, tag="pss")
            ksl = slice(kt_i * P, (kt_i + 1) * P)
            for qb in range(2):
                nc.tensor.matmul(
                    ps[:, qb, :],
                    lhsT=kt[0:D, ct, ksl],
                    rhs=qt[0:D, ct, qb * 512:(qb + 1) * 512],
                    start=True, stop=True)
                nc.tensor.matmul(
                    ps[:, 2 + qb, :],
                    lhsT=kt[D:P, ct, ksl],
                    rhs=qt[D:P, ct, qb * 512:(qb + 1) * 512],
                    start=True, stop=True)
            # psum layout [Aq0, Aq1, Bq0, Bq1] matches ptp [head][1024]
            nc.scalar.activation(
                ptp[:, :, kt_i, :],
                ps.rearrange("p (h q) n -> p h (q n)", h=2),
                EXP, scale=0.125)

        def ctx_mm_span(h, qb, psx, lo, hi):
            ptp = pt_of[h // 2]
            for kt_i in range(lo, hi):
                nc.tensor.matmul(
                    psx[0:D + 1, :],
                    lhsT=vaug[:, kt_i, h, :],
                    rhs=ptp[:, h % 2, kt_i, qb * 512:(qb + 1) * 512],
                    start=(kt_i == 0), stop=(kt_i == 7))

        def emit_ctx_mm(h, qb, lo=0, hi=8, psx=None):
            """Unnormalized ctx^T [65, 512] for (head, q-half); on the last
            chunk it is evacuated to fp16 SBUF (the only DVE op the PE chain
            waits on)."""
            if psx is None:
                psx = psc.tile([P, 512], f32, tag="psc")
            ctx_mm_span(h, qb, psx, lo, hi)
            if hi < 8:
                return psx
            cts = rpool.tile([D + 1, 512], f16, tag="cts")
            nc.vector.tensor_copy(cts, psx[0:D + 1, :])
            return cts

        def emit_ctx_tr(h, qb, cts, oc):
            """Transpose [65,512] -> [q, d+1], per-partition recip of the
            denominator column, one broadcast multiply, DMA per head."""
            ps_t = pst.tile([P, 4, D + 2], f16, tag="pst")
            for j in range(4):
                nc.tensor.transpose(
                    ps_t[:, j, 0:D + 1], cts[:, j * P:(j + 1) * P],
                    ident[0:D + 1, 0:D + 1])
            rcp = rpool.tile([P, 4], f32, tag="rcp")
            nc.vector.reciprocal(rcp, ps_t[:, :, D])
            rcp_b, _ = bass.broadcast_tensor_aps(
                rcp.rearrange("p (f o) -> p f o", o=1),
                ps_t[:, :, 0:D])
            nc.vector.tensor_tensor(
                oc[:, qb, :, :], ps_t[:, :, 0:D], rcp_b,
                mybir.AluOpType.mult)
            if qb == 1:
                nc.sync.dma_start(out=out_d[h], in_=oc)

        def ctx_pair_fillers(ha):
            """PE-slot chunks for the ctx of heads (ha, ha+1): the transpose
            of one (h,qb) overlaps the ctx matmuls of the next."""
            steps = [(ha, 0), (ha, 1), (ha + 1, 0), (ha + 1, 1)]
            state = {}

            def mk_mm(h, qb):
                def f():
                    if qb == 0:
                        state[h] = opool.tile([P, 2, 4, D], f16, tag="oc",
                                              name=f"oc{h}")
                    state[(h, qb)] = emit_ctx_mm(h, qb)
                return f

            def mk_tr(h, qb):
                def f():
                    emit_ctx_tr(h, qb, state.pop((h, qb)), state[h])
                return f

            out = []
            for i, (h, qb) in enumerate(steps):
                fs = [mk_mm(h, qb)]
                if i > 0:
                    fs.insert(0, mk_tr(*steps[i - 1]))
                out.append(fs)
            out.append([mk_tr(*steps[-1])])
            return out  # 5 filler slots

        def run_fillers(fillers, lo, hi):
            for fs in fillers[lo:hi]:
                for f in fs:
                    f()

        # ---- software pipeline over head pairs ----
        # ct0 projections accumulate 4 chains hcc-major in the (still idle)
        # 4-bank scores tile, so every chain tracks the X stream and all
        # finish right after the last X chunk lands
        ps0 = pss.tile([P, 4, 512], f32, tag="pss", name="ps0")
        for hcc in range(8):
            for c4 in range(4):
                wi, sb = c4 // 2, c4 % 2
                nc.tensor.matmul(
                    ps0[:, c4, :],
                    lhsT=wqk_s[:, 0, wi, hcc, :],
                    rhs=xt[:, hcc, sb * 512:(sb + 1) * 512],
                    start=(hcc == 0), stop=(hcc == 7))
        for c4 in range(4):
            wi, sb = c4 // 2, c4 % 2
            b_s, dst = ((bqs, qt), (bks, kt))[wi]
            nc.vector.tensor_scalar_add(
                dst[:, 0, sb * 512:(sb + 1) * 512], ps0[:, c4, :],
                b_s[:, 0:1])

        part3 = {}
        for ct in range(4):
            fillers = []
            if ct == 0:
                fillers += [[lambda wi=wi, sb=sb: emit_qtkt_chunk(1, wi, sb)]
                            for wi in range(2) for sb in range(2)]
                fillers += [[lambda st=st: emit_v_chunk(st)] for st in range(6)]
            else:
                if ct == 1:
                    fillers += [[lambda st=st: emit_v_chunk(st)]
                                for st in range(6, 8)]
                prev = ctx_pair_fillers(2 * (ct - 1))
                if ct == 2:
                    fillers += prev[0:3]
                    spill = prev[3:]
                elif ct == 3:
                    fillers = spill + fillers
                    fillers += prev
                else:
                    fillers += prev
                if ct < 3:
                    fillers += [[lambda wi=wi, sb=sb, c=ct + 1:
                                 emit_qtkt_chunk(c, wi, sb)]
                                for wi in range(2) for sb in range(2)]
                else:
                    for key in ((6, 0), (6, 1), (7, 0)):
                        part3[key] = []
                    fillers += [
                        [lambda: part3[(6, 0)].append(
                            emit_ctx_mm(6, 0, 0, 6))],
                        [lambda: part3[(6, 1)].append(
                            emit_ctx_mm(6, 1, 0, 6))],
                        [lambda: part3[(7, 0)].append(
                            emit_ctx_mm(7, 0, 0, 7))],
                    ]
            n = len(fillers)
            pos = 0
            for kt_i in range(8):
                emit_scores4(ct, kt_i)
                nxt = (kt_i + 1) * n // 8
                run_fillers(fillers, pos, nxt)
                pos = nxt
        # tail: finish head 6 (kt6-7 + evacuate), then head 7 in full
        oc6 = opool.tile([P, 2, 4, D], f16, tag="oc", name="oc6")
        oc7 = opool.tile([P, 2, 4, D], f16, tag="oc", name="oc7")
        cts = {}
        for qb in range(2):
            cts[(6, qb)] = emit_ctx_mm(6, qb, 6, 8, part3[(6, qb)][0])
        emit_ctx_tr(6, 0, cts[(6, 0)], oc6)
        cts[(7, 0)] = emit_ctx_mm(7, 0, 7, 8, part3[(7, 0)][0])
        emit_ctx_tr(6, 1, cts[(6, 1)], oc6)
        cts[(7, 1)] = emit_ctx_mm(7, 1)
        emit_ctx_tr(7, 0, cts[(7, 0)], oc7)
        emit_ctx_tr(7, 1, cts[(7, 1)], oc7)

    _split_excess_waits(nc, mybir)
    return nc


def _get_nc():
    if "nc" not in _CACHE:
        _CACHE["nc"] = _build()
    return _CACHE["nc"]


def _in_maps(inputs):
    hs = np.ascontiguousarray(np.asarray(inputs["hidden_states"], dtype=np.float32))
    maps = []
    for c in range(NCORES):
        b, g = c // 2, c % 2
        sl = slice(g * CW, (g + 1) * CW)
        m = {"x": np.ascontiguousarray(hs[b].T).astype(np.float16)}
        m["wv"] = np.ascontiguousarray(
            np.asarray(inputs["Wv"], dtype=np.float32)[:, sl]).astype(np.float16)
        wq4 = np.asarray(inputs["Wq"], dtype=np.float32)[:, sl].reshape(
            8, P, 4, P).transpose(2, 1, 0, 3)
        wk4 = np.asarray(inputs["Wk"], dtype=np.float32)[:, sl].reshape(
            8, P, 4, P).transpose(2, 1, 0, 3)
        m["wqk"] = np.ascontiguousarray(
            np.stack([wq4, wk4], axis=2)).astype(np.float16)
        for nm, bk in (("bq", "bq"), ("bk", "bk")):
            m[nm] = np.ascontiguousarray(
                np.asarray(inputs[bk], dtype=np.float32)[sl].reshape(4, P).T)
        m["bvb"] = np.ascontiguousarray(np.broadcast_to(
            np.asarray(inputs["bv"], dtype=np.float32)[sl], (P, CW)))
        m["id16"] = np.eye(P, dtype=np.float16)
        maps.append(m)
    return maps


def run(inputs, **spmd_kwargs):
    """Run on 8 cores; returns (full_output, BassKernelResults)."""
    from concourse.bass_utils import run_bass_kernel_spmd
    nc = _get_nc()
    res = run_bass_kernel_spmd(nc, _in_maps(inputs), list(range(NCORES)),
                               **spmd_kwargs)
    out = np.empty((B, S, H), dtype=np.float32)
    for c in range(NCORES):
        b, g = c // 2, c % 2
        a = res.results[c]["out"]  # [h, p, qb, j, d]
        out[b, :, g * CW:(g + 1) * CW] = (
            a.transpose(2, 3, 1, 0, 4).reshape(S, CW).astype(np.float32))
    return out, res


def kernel(**inputs):
    out, _ = run(inputs)
    return out
